# revision 9
# baseline (speedup 1.0000x reference)
"""Trainium2 Bass kernel for nn_MixBlock (StyleGAN2-style modulated conv block).

reference semantics:
  x:[8,256,64,64] -> bilinear up x2 -> modconv(3x3, s1) -> lrelu(0.2)
  -> modconv(3x3, s2) -> lrelu(0.2) -> y:[8,256,128,128]

Sharding: data-parallel over batch, 1 sample per NeuronCore (8 cores).
Weights / style-linear params replicated to every core.

Per-core device program:
  - style: s[c] = sum_l ws[c,l]*istyle[l] + bs[c];  m = 1+s
  - fold modulation into weights: wT[c, :] *= m[c]  (wT pre-transposed on host
    to [C, (kh kw) O] so matmul lhsT tiles are contiguous)
  - demod: d[o] = 1/sqrt(sum_c r[c,o]*m[c]^2 + eps) via 2 tiny PE matmuls
    (r[c,o] = sum_t w[o,c,t]^2 precomputed on host - sample independent)
  - bilinear upsample x2 materialized in 16-output-row band tiles (18 up-rows
    with 1px zero border) using scalar_tensor_tensor 3*a+b ops; bands hold
    16*x_up, the 1/16 is folded into d1.
  - conv = 9 taps x 2 C-chunks float32r matmuls (N=512 = 4 output rows per
    PSUM group, 4 groups per band tile) accumulated in PSUM;
    drain = 0.8*relu(d*psum) [ACT] + 0.2*d*psum [DVE] = lrelu(d*psum);
    conv1 result y1 staged in a DRAM scratch tile, conv2 reads it back in
    18-row band tiles (split DMAs for queue parallelism).

float32r: fp32-width PE dtype at 1 cycle/row (vs 4 for plain fp32), tf32-like
precision (~2e-4 rel per K=128 matmul; full-kernel rel err ~3e-4).
KERNEL_MMDT=f32 env flips the conv path back to exact fp32 (4x slower PE).
"""

import os
import numpy as np
from contextlib import ExitStack

import concourse.bass as bass
import concourse.bacc as bacc
import concourse.mybir as mybir
import concourse.tile as tile

F32 = mybir.dt.float32
F32R = mybir.dt.float32r
MM_DT = F32R if os.environ.get("KERNEL_MMDT", "f32r") == "f32r" else F32
MULT = mybir.AluOpType.mult
ADD = mybir.AluOpType.add
EPS = 1e-8
LEAK = 0.2

C = 256  # channels (conv1 in = conv1 out = conv2 in/out = 256)
G = 2    # C partition chunks
H = W = 64
H2 = W2 = 128
NTAP = 9
BAND = 4            # output rows per PSUM group (N = BAND*W2 = 512)
BANDT = 16          # output rows per band tile (4 PSUM groups)
NBT = H2 // BANDT   # band tiles per image


def _memset0(nc, ap):
    # walrus rejects InstMemset on float32r APs -> relabel as plain f32
    if ap.dtype == F32R:
        ap = ap.bitcast(F32)
    nc.vector.memset(ap, 0.0)


def _emit_vertical(nc, x, tmp, rb):
    """tmp[:, t, :] = 4 * up_v[rb-1+t]  for t=0..17 (vertical bilinear pass).

    up_v[u]: even u=2i -> 0.75*x[i]+0.25*x[i-1] (clamped);
             odd u=2i+1 -> 0.75*x[i]+0.25*x[i+1] (clamped);
    u=-1 / u=128 are conv zero-pad rows. rb is a multiple of 16, so even-u
    rows sit at odd slots t.
    """
    stt = nc.vector.scalar_tensor_tensor
    i = rb // 2
    if rb == 0:
        _memset0(nc, tmp[:, 0:1, :])                                # u=-1 pad
        nc.vector.tensor_scalar_mul(tmp[:, 1:2, :], x[:, 0:1, :], 4.0)  # u=0
        # odd u=1..15 -> slots 2,4..16 (8 rows), i=0..7
        stt(tmp[:, 2:17:2, :], x[:, 0:8, :], 3.0, x[:, 1:9, :], MULT, ADD)
        # even u=2..16 -> slots 3,5..17 (8 rows), i=1..8
        stt(tmp[:, 3:18:2, :], x[:, 1:9, :], 3.0, x[:, 0:8, :], MULT, ADD)
    elif rb == H2 - BANDT:  # rb=112: u=111..128, i=56..63
        # odd u=111..125 -> slots 0,2..14 (8 rows), i=55..62
        stt(tmp[:, 0:15:2, :], x[:, 55:63, :], 3.0, x[:, 56:64, :], MULT, ADD)
        # even u=112..126 -> slots 1,3..15 (8 rows), i=56..63
        stt(tmp[:, 1:16:2, :], x[:, 56:64, :], 3.0, x[:, 55:63, :], MULT, ADD)
        nc.vector.tensor_scalar_mul(tmp[:, 16:17, :], x[:, 63:64, :], 4.0)  # u=127
        _memset0(nc, tmp[:, 17:18, :])                              # u=128 pad
    else:
        # even u=rb..rb+16 -> slots 1,3..17 (9 rows), in0=x[i..i+8]
        stt(tmp[:, 1:18:2, :], x[:, i:i + 9, :], 3.0, x[:, i - 1:i + 8, :],
            MULT, ADD)
        # odd u=rb-1..rb+15 -> slots 0,2..16 (9 rows), in0=x[i-1..i+7]
        stt(tmp[:, 0:17:2, :], x[:, i - 1:i + 8, :], 3.0, x[:, i:i + 9, :],
            MULT, ADD)


def _emit_horizontal(nc, tmp, band):
    """band[:, t, 1+j] = 4 * up_h(tmp)[j]; cols 0 and 129 zero-padded."""
    stt = nc.vector.scalar_tensor_tensor
    _memset0(nc, band[:, :, 0:130:129])
    # even out cols 2j (j=1..63) at padded pos 3,5..127
    stt(band[:, :, 3:128:2], tmp[:, :, 1:64], 3.0, tmp[:, :, 0:63], MULT, ADD)
    # odd out cols 2j+1 (j=0..62) at padded pos 2,4..126
    stt(band[:, :, 2:127:2], tmp[:, :, 0:63], 3.0, tmp[:, :, 1:64], MULT, ADD)
    nc.vector.tensor_scalar_mul(band[:, :, 1:2], tmp[:, :, 0:1], 4.0)
    nc.vector.tensor_scalar_mul(band[:, :, 128:129], tmp[:, :, 63:64], 4.0)


def build_nc(bench_loop=0, no_ydma=False):
    nc = bacc.Bacc("TRN2", target_bir_lowering=False, debug=False)

    x_in = nc.dram_tensor("x", [G, 128, H, W], MM_DT, kind="ExternalInput")
    ist_in = nc.dram_tensor("istyle", [1, 512], F32, kind="ExternalInput")
    ws_in = [nc.dram_tensor(f"ws{i}", [G, 128, 512], F32, kind="ExternalInput")
             for i in (1, 2)]
    bs_in = [nc.dram_tensor(f"bs{i}", [G, 128, 1], F32, kind="ExternalInput")
             for i in (1, 2)]
    wt_in = [nc.dram_tensor(f"w{i}t", [G, 128, NTAP * C], MM_DT,
                            kind="ExternalInput") for i in (1, 2)]
    r_in = [nc.dram_tensor(f"r{i}", [G, 128, C], F32, kind="ExternalInput")
            for i in (1, 2)]
    y_out = nc.dram_tensor("y", [G, 128, H2, W2], F32, kind="ExternalOutput")

    with tile.TileContext(nc) as tc, ExitStack() as ctx:
        const = ctx.enter_context(tc.tile_pool(name="const", bufs=1))
        dram = ctx.enter_context(tc.tile_pool(name="dram", bufs=1, space="DRAM"))
        bandp = ctx.enter_context(tc.tile_pool(name="bandp", bufs=2))
        tmpp = ctx.enter_context(tc.tile_pool(name="tmpp", bufs=2))
        outp = ctx.enter_context(tc.tile_pool(name="outp", bufs=4))
        psum = ctx.enter_context(tc.tile_pool(name="psum", bufs=6, space="PSUM"))
        psd = ctx.enter_context(tc.tile_pool(name="psd", bufs=2, space="PSUM"))

        # ---------------- constants in ----------------
        xs = []
        for g in range(G):
            t = const.tile([128, H, W], MM_DT, name=f"xs{g}")
            nc.sync.dma_start(t[:], x_in[g])
            xs.append(t)
        wts, rs, wss, bss = [], [], [], []
        for i in range(2):
            wts.append([])
            rs.append([])
            wss.append([])
            bss.append([])
            for g in range(G):
                t = const.tile([128, NTAP * C], MM_DT, name=f"w{i}t{g}")
                nc.sync.dma_start(t[:], wt_in[i][g])
                wts[i].append(t)
                t = const.tile([128, C], F32, name=f"r{i}_{g}")
                nc.sync.dma_start(t[:], r_in[i][g])
                rs[i].append(t)
                t = const.tile([128, 512], F32, name=f"ws{i}_{g}")
                nc.sync.dma_start(t[:], ws_in[i][g])
                wss[i].append(t)
                t = const.tile([128, 1], F32, name=f"bs{i}_{g}")
                nc.sync.dma_start(t[:], bs_in[i][g])
                bss[i].append(t)
        istb = const.tile([128, 512], F32, name="istb")
        nc.sync.dma_start(istb[:], ist_in[0:1, :].to_broadcast([128, 512]))
        epst = const.tile([128, 1], F32, name="epst")
        nc.vector.memset(epst[:], EPS)

        # ---------------- styles, weight modulation, demod ----------------
        dmod = [[None] * G for _ in range(2)]  # demod scale d per o-chunk
        for i in range(2):
            msq = []
            for g in range(G):
                junk = tmpp.tile([128, 512], F32, name="junk")
                sr = const.tile([128, 1], F32, name=f"sr{i}{g}")
                # (tensor_tensor_reduce w/ accum_out hard-crashes the exec
                # unit on this runtime -> use mul + reduce instead)
                nc.vector.tensor_mul(junk[:], wss[i][g][:], istb[:])
                nc.vector.tensor_reduce(sr[:], junk[:],
                                        axis=mybir.AxisListType.X, op=ADD)
                m = const.tile([128, 1], F32, name=f"m{i}{g}")
                nc.vector.scalar_tensor_tensor(m[:], sr[:], 1.0, bss[i][g][:],
                                               ADD, ADD)
                nc.vector.tensor_scalar_mul(wts[i][g][:], wts[i][g][:], m[:])
                mq = const.tile([128, 1], F32, name=f"mq{i}{g}")
                nc.vector.tensor_mul(mq[:], m[:], m[:])
                msq.append(mq)
            for oh in range(G):
                pd = psd.tile([128, 1], F32, name="pd")
                for g in range(G):
                    nc.tensor.matmul(pd[:], rs[i][g][:, oh * 128:(oh + 1) * 128],
                                     msq[g][:], start=(g == 0), stop=(g == G - 1))
                sq = const.tile([128, 1], F32, name=f"sq{i}{oh}")
                nc.scalar.activation(sq[:], pd[:],
                                     mybir.ActivationFunctionType.Sqrt,
                                     bias=epst[:])
                dv = const.tile([128, 1], F32, name=f"dv{i}{oh}")
                nc.vector.reciprocal(dv[:], sq[:])
                if i == 0:
                    nc.vector.tensor_scalar_mul(dv[:], dv[:], 1.0 / 16.0)
                dmod[i][oh] = dv

        # y1 ring in SBUF: 21 slots of 130-wide rows per o-chunk.
        # slot s (s<16) holds y1 row u with u%16==s; rows with u%16<4 are
        # duplicated at slot 16+(u%16), and row u%16==4 at slot 20, so every
        # conv2 group reads 6 consecutive slots: sb=(r-1)%16 -> sb..sb+5.
        ring = []
        for og in range(G):
            t = const.tile([128, 21, 130], MM_DT, name=f"ring{og}")
            _memset0(nc, t[:])
            ring.append(t)

        loop_ctx = tc.For_i(0, bench_loop, 1) if bench_loop else None
        if loop_ctx is not None:
            loop_ctx.__enter__()

        def conv_psum(ps, wconv, bands, og, base):
            """18 accumulating matmuls; bands[g] slot base holds input row
            r-1, output row r+k tap dy reads slot base+1+k+dy."""
            k = 0
            for dy in (-1, 0, 1):
                for dx in (-1, 0, 1):
                    t = (dy + 1) * 3 + (dx + 1)
                    off = t * C + og * 128
                    for g in range(G):
                        nc.tensor.matmul(
                            ps[:], wconv[g][:, off:off + 128],
                            bands[g][:, base + 1 + dy:base + 5 + dy,
                                     1 + dx:129 + dx],
                            start=(k == 0), stop=(k == 2 * NTAP - 1))
                        k += 1

        def prelu_drain(ps, i, og, out_ap, cols):
            """out = lrelu(d*ps) in a single ACT op (Prelu is exact on hw)."""
            nc.scalar.activation(out_ap, ps[:, 0:cols],
                                 mybir.ActivationFunctionType.Prelu,
                                 scale=dmod[i][og][:], alpha=LEAK)

        c1_bands = [None, None]

        def emit_c1_group(j):
            rb, sub = (j // 4) * BANDT, j % 4
            if sub == 0:
                for g in range(G):
                    tmp = tmpp.tile([128, BANDT + 2, W], MM_DT, name=f"tmp{g}")
                    _emit_vertical(nc, xs[g], tmp, rb)
                    band = bandp.tile([128, BANDT + 2, 130], MM_DT,
                                      name=f"band{g}")
                    _emit_horizontal(nc, tmp, band)
                    c1_bands[g] = band
            r = rb + sub * BAND
            p = r % 16
            for og in range(G):
                ps = psum.tile([128, BAND * W2], F32, name="ps")
                conv_psum(ps, wts[0], c1_bands, og, sub * BAND)
                # drain straight into the ring (primary slots)
                prelu_drain(ps, 0, og, ring[og][:, p:p + 4, 1:129], 512)
                if p == 0:    # duplicate rows r..r+3 at slots 16..19
                    prelu_drain(ps, 0, og, ring[og][:, 16:20, 1:129], 512)
                elif p == 4:  # duplicate row r at slot 20
                    prelu_drain(ps, 0, og, ring[og][:, 20:21, 1:129], 128)

        def emit_c2_group(j):
            r = j * BAND
            sb = (r - 1) % 16
            for og in range(G):
                ps = psum.tile([128, BAND * W2], F32, name="ps")
                conv_psum(ps, wts[1], ring, og, sb)
                o = outp.tile([128, BAND * W2], F32, name="o2", bufs=6)
                prelu_drain(ps, 1, og, o[:], 512)
                if not no_ydma:
                    nc.sync.dma_start(y_out[og, :, r:r + BAND, :], o[:])

        NG = H2 // BAND  # 32 PSUM groups per conv
        for j in range(NG):
            emit_c1_group(j)
            if j >= 2:
                emit_c2_group(j - 2)
        # rows 128.. are the conv zero-pad: slot 16 (read as row 128 by the
        # last group) was left holding stale dup rows -> zero it.  Safe here:
        # its last legitimate reader (group r=112) is already emitted.
        for og in range(G):
            _memset0(nc, ring[og][:, 16:17, :])
        emit_c2_group(NG - 2)
        emit_c2_group(NG - 1)

        if loop_ctx is not None:
            loop_ctx.__exit__(None, None, None)

    nc.compile()
    return nc


def _host_prep(x, istyle, ws1, bs1, conv1_w, ws2, bs2, conv2_w):
    """Per-core input maps. Sample-independent layout transforms only."""
    w1t = np.ascontiguousarray(
        conv1_w.transpose(1, 2, 3, 0).reshape(G, 128, NTAP * C))
    w2t = np.ascontiguousarray(
        conv2_w.transpose(1, 2, 3, 0).reshape(G, 128, NTAP * C))
    r1 = np.ascontiguousarray(
        (conv1_w * conv1_w).sum(axis=(2, 3)).T.reshape(G, 128, C))
    r2 = np.ascontiguousarray(
        (conv2_w * conv2_w).sum(axis=(2, 3)).T.reshape(G, 128, C))
    ws1r = np.ascontiguousarray(ws1.reshape(G, 128, 512))
    ws2r = np.ascontiguousarray(ws2.reshape(G, 128, 512))
    bs1r = np.ascontiguousarray(bs1.reshape(G, 128, 1))
    bs2r = np.ascontiguousarray(bs2.reshape(G, 128, 1))
    in_maps = []
    for b in range(8):
        in_maps.append({
            "x": np.ascontiguousarray(x[b].reshape(G, 128, H, W)),
            "istyle": np.ascontiguousarray(istyle[b].reshape(1, 512)),
            "ws1": ws1r, "bs1": bs1r, "w1t": w1t, "r1": r1,
            "ws2": ws2r, "bs2": bs2r, "w2t": w2t, "r2": r2,
        })
    return in_maps


_NC_CACHE = None
_LAST_RESULT = None  # BassKernelResults, for test harness introspection


def kernel(x, istyle, ws1, bs1, conv1_w, ws2, bs2, conv2_w):
    global _NC_CACHE, _LAST_RESULT
    from concourse.bass_utils import run_bass_kernel_spmd

    x = np.asarray(x, dtype=np.float32)
    istyle = np.asarray(istyle, dtype=np.float32)
    ws1 = np.asarray(ws1, dtype=np.float32)
    bs1 = np.asarray(bs1, dtype=np.float32)
    conv1_w = np.asarray(conv1_w, dtype=np.float32)
    ws2 = np.asarray(ws2, dtype=np.float32)
    bs2 = np.asarray(bs2, dtype=np.float32)
    conv2_w = np.asarray(conv2_w, dtype=np.float32)

    if _NC_CACHE is None:
        _NC_CACHE = build_nc()
    nc = _NC_CACHE

    in_maps = _host_prep(x, istyle, ws1, bs1, conv1_w, ws2, bs2, conv2_w)
    trace = bool(int(os.environ.get("KERNEL_TRACE", "0")))
    res = run_bass_kernel_spmd(nc, in_maps, core_ids=list(range(8)), trace=trace)
    _LAST_RESULT = res
    out = np.stack([res.results[b]["y"].reshape(C, H2, W2) for b in range(8)])
    return out

